# revision 37
# baseline (speedup 1.0000x reference)
"""Trainium2 Bass kernel for LorentzMultiheadAttention (B=2, N=2048, H=8, D=64, E=512).

Sharding: 8 cores = 2 batches x 4 query-quarters. Core c handles batch b=c//4
and queries [512*(c%4), 512*(c%4+1)) for ALL 8 heads. K/V projections are
recomputed on each core of a batch group (cheaper than an inter-core
ReduceScatter) so the kernel has NO collectives: per-head centroids, the head
mean, and the second centroid are all core-local.

Structure: a 4-stage pipeline over head-pairs. For each head-pair hp:
project K/V -> lift -> one xbar-transpose of V -> 16 attention iterations
(2 score MMs -> EXP[128,1024] -> 2 PV MMs). The next head-pair's projections
are scheduled into the PE slack under the current pair's (ACT-bound) EXP
stream. PSUM: 4 score banks + 2 PV banks (alternating per hp) + 2 utility
banks (projections / lift sums / output transposes) = 8.

ACT table-set discipline: sqrt(x) is computed as exp(0.5*ln(x)) and
1/sqrt(|x|) as exp(-0.5*ln(-x)) so every ACT instruction (lifts, attention
EXPs, centroid scales) draws from the natural_log_exp table set -- no
mid-stream ACT_TABLE_LOADs.

Layout tricks:
- The [128,2048]->[128,16,128] DMA xbar transpose delivers transposed row r
  (= key index) to partition r%128, free tile r//128 -- exactly the
  natural-layout V that PV matmuls need, in ONE instruction per head-pair.
- Lift time rows: accumulating mask-variant matmuls pack all of a head-pair's
  sum(x_spatial^2) rows (K and V, 4 column chunks, even/odd head) into one
  PSUM bank at rows 0..15, so a single 16-lane Ln+Exp computes every
  t = sqrt(1+s); small SBUF->SBUF DMAs scatter the two time rows of each
  target to partitions {0,64}.

Math notes:
- The Lorentz centroid sqrt(C)*x/sqrt(|<x,x>_L|) is scale-invariant, so the
  softmax denominator and the mean-over-heads divide both cancel; PV feeds
  unnormalized sum_m exp(att)*v into the centroid.
- The Lorentz sign is folded by negating K weights on the host:
  scores S' = t_q*t_k - q_s.k_s = -<q,k>_L and softmax weights are
  exp(-(2/s)*S' + (2/s + bias)). No max-subtraction: |att| <= ~3.
"""

import os
import sys

for _p in ("/opt/trn_rl_repo", "/root/.axon_site/_ro/trn_rl_repo"):
    if os.path.isdir(_p) and _p not in sys.path:
        sys.path.insert(0, _p)

import numpy as np

import concourse.bacc as bacc
import concourse.bass as bass
import concourse.mybir as mybir
import concourse.tile as tile

B = 2
N = 2048
H = 8
D = 64
E = 512
DM1 = D - 1  # 63
P = 128
N_CORES = 8
QB = N // 4  # 512 queries per core
NHP = 4  # head-pairs per core

F32 = mybir.dt.float32
BF16 = mybir.dt.bfloat16
I32 = mybir.dt.int32
EXP = mybir.ActivationFunctionType.Exp
SQRT = mybir.ActivationFunctionType.Sqrt
ADD = mybir.AluOpType.add
SUB = mybir.AluOpType.subtract
MULT = mybir.AluOpType.mult
SHR = mybir.AluOpType.logical_shift_right
QMAGIC = 0x5F3759DF


def _emit(tc, nc, io, scale_val, bias_val):
    from contextlib import ExitStack

    ctx = ExitStack()
    with ctx:
        consts = ctx.enter_context(tc.tile_pool(name="consts", bufs=1))
        sb = ctx.enter_context(tc.tile_pool(name="sb", bufs=1))
        scr = ctx.enter_context(tc.tile_pool(name="scr", bufs=2))
        pP = ctx.enter_context(tc.tile_pool(name="pP", bufs=6))
        psU = ctx.enter_context(tc.tile_pool(name="psU", bufs=2, space="PSUM"))
        psPV = ctx.enter_context(tc.tile_pool(name="psPV", bufs=1, space="PSUM"))
        psS = ctx.enter_context(tc.tile_pool(name="psS", bufs=2, space="PSUM"))

        # ---- PE warm-up: the HAM clock-gate starts at 1.2 GHz and needs
        # ~3.4us of sustained matmul activity to reach 2.4 GHz. Input DMAs
        # take ~12us to land, so burn that dead time on scratch matmuls and
        # the real projections start warm.
        warm = sb.tile([P, 512], BF16, name="warm")
        nc.vector.memset(warm[:], 0.5)
        for _ in range(14):
            wps = psU.tile([P, 512], F32, tag="u", name="warmps")
            nc.tensor.matmul(
                wps[:], lhsT=warm[:, 0:P], rhs=warm[:], start=True, stop=True
            )

        # ---- constants / weights (Q-path inputs first so Q proj starts early)
        ident2 = consts.tile([P, 64], F32)
        nc.sync.dma_start(ident2[:], io["ident2"].ap())
        # mask32[:, j, :]: lift-mask variant writing head-sums to rows {2j,2j+1}
        mask32 = consts.tile([P, 16, 32], BF16)
        nc.sync.dma_start(mask32[:], io["mask32"].ap())

        w_sb = {}
        b_sb = {}

        def load_w(nm):
            w = consts.tile([P, 4, 4, P], BF16, name=f"{nm}_sb")
            nc.sync.dma_start(w[:], io[nm].ap())
            w_sb[nm] = w
            bn = "b" + nm[1]
            bt = consts.tile([P, 4], F32, name=f"{bn}_sb")
            nc.sync.dma_start(bt[:], io[bn].ap())
            b_sb[bn] = bt

        load_w("wq")
        xq = sb.tile([P, 4, QB], BF16)
        nc.sync.dma_start(xq[:], io["xq_t"].ap())

        ebias = consts.tile([P, 1], F32)
        nc.vector.memset(ebias[:], 2.0 / scale_val + bias_val)

        qsT = sb.tile([P, NHP, QB], BF16)
        ksT = sb.tile([P, NHP, N], BF16)
        vT = sb.tile([P, NHP, N], BF16)
        v_nat = sb.tile([P, 16, NHP, P], BF16)  # [p, mc, hp, 2h*64]; key=mc*128+p

        IDENT = mybir.ActivationFunctionType.Identity

        def project(dst_sl, x_sl, w, pt, bias, qcs, copy_on_act=False):
            for qc in qcs:
                ps = psU.tile([P, 512], F32, tag="u", name="proj")
                for ec in range(4):
                    nc.tensor.matmul(
                        ps[:],
                        lhsT=w[:, ec, pt, :],
                        rhs=x_sl[:, ec, qc * 512 : (qc + 1) * 512],
                        start=(ec == 0),
                        stop=(ec == 3),
                    )
                dst = dst_sl[:, qc * 512 : (qc + 1) * 512]
                if copy_on_act:
                    # ACT is idle before the EXP stream starts; Identity is in
                    # every table set so this forces no ACT_TABLE_LOAD.
                    nc.scalar.activation(dst, ps[:], IDENT, bias=bias)
                else:
                    nc.vector.tensor_tensor(
                        dst, ps[:], bias.to_broadcast((P, 512)), ADD
                    )

        qmagic = consts.tile([P, 1], I32)
        nc.vector.memset(qmagic[:], QMAGIC)

        def rsqrt_dve(u, tag, iters=1):
            """1/sqrt(u) on the vector engine: Quake seed + Newton steps.
            u: f32 SBUF AP. Returns an f32 tile of the same shape. Keeps the
            ACT engine (and its exp-table) untouched by the lifts."""
            shp = list(u.shape)
            y = scr.tile(shp, F32, tag=f"{tag}y", name="qk_y")
            sh = scr.tile(shp, I32, tag=f"{tag}i", name="qk_i")
            nc.vector.tensor_scalar(sh[:], u.bitcast(I32), 1, None, SHR)
            nc.vector.tensor_tensor(
                y[:].bitcast(I32),
                qmagic[0 : shp[0], :].to_broadcast(tuple(shp)),
                sh[:],
                SUB,
            )
            z = scr.tile(shp, F32, tag=f"{tag}z", name="qk_z")
            for _ in range(iters):
                nc.vector.tensor_tensor(z[:], y[:], y[:], MULT)
                nc.vector.tensor_tensor(z[:], u, z[:], MULT)
                nc.vector.tensor_scalar(z[:], z[:], -0.5, 1.5, MULT, ADD)
                nc.vector.tensor_tensor(y[:], y[:], z[:], MULT)
            return y

        def lift_times(dst, nrm_ps, tag):
            """dst (bf16) = sqrt(1 + nrm_ps) via u*rsqrt(u), DVE-only."""
            shp = list(nrm_ps.shape)
            u = scr.tile(shp, F32, tag=f"{tag}u", name="qk_u")
            nc.vector.tensor_scalar(u[:], nrm_ps, 1.0, None, ADD)
            y = rsqrt_dve(u[:], tag)
            nc.vector.tensor_tensor(dst, u[:], y[:], MULT)

        # ---- Q projection + lift (all 4 head-pairs) ----
        for hp in range(NHP):
            project(
                qsT[:, hp, :], xq, w_sb["wq"], hp, b_sb["bq"][:, hp : hp + 1], [0],
                copy_on_act=True,
            )
        qsq = sb.tile([P, NHP, QB], BF16)
        nc.vector.tensor_tensor(qsq[:], qsT[:], qsT[:], MULT)
        qnrm = psU.tile([8, 512], F32, tag="u", name="qnrm")
        for hp in range(NHP):
            nc.tensor.matmul(
                qnrm[:],
                lhsT=mask32[:, hp, 0:8],
                rhs=qsq[:, hp, :],
                start=(hp == 0),
                stop=(hp == NHP - 1),
            )
        qt_s = scr.tile([8, 512], BF16, tag="qts", bufs=1)
        nc.scalar.activation(qt_s[:], qnrm[:], SQRT, bias=1.0, scale=1.0)
        for hp in range(NHP):
            nc.sync.dma_start(qsT[0:65:64, hp, :], qt_s[2 * hp : 2 * hp + 2, :])

        # source-side inputs (after the Q chain so xq lands first); xs arrives
        # in 4 column chunks so the first K projection starts ASAP.
        load_w("wk")
        load_w("wv")
        xs = sb.tile([P, 4, N], BF16)
        for qc in range(4):
            nc.sync.dma_start(
                xs[:, :, qc * 512 : (qc + 1) * 512], io[f"xs{qc}"].ap()
            )

        act_scale = -2.0 / scale_val
        pv_tiles = {}

        def lift_one_act(srcdst, hp):
            """Pre-stream lift of a single ksT/vT head-pair slice via ACT sqrt
            (ACT is idle before the EXP stream; 0.7us vs ~5.5us DVE Quake)."""
            nrm = psU.tile([8, 512], F32, tag="u", name="nrm")
            sq = scr.tile([P, N], BF16, tag="ksq")
            nc.vector.tensor_tensor(sq[:], srcdst[:, hp, :], srcdst[:, hp, :], MULT)
            for qc in range(4):
                nc.tensor.matmul(
                    nrm[:],
                    lhsT=mask32[:, qc, 0:8],
                    rhs=sq[:, qc * 512 : (qc + 1) * 512],
                    start=(qc == 0),
                    stop=(qc == 3),
                )
            kvt = scr.tile([8, 512], BF16, tag="kvt8")
            nc.scalar.activation(kvt[:], nrm[:], SQRT, bias=1.0, scale=1.0)
            for qc in range(4):
                nc.sync.dma_start(
                    srcdst[0:65:64, hp, qc * 512 : (qc + 1) * 512],
                    kvt[2 * qc : 2 * qc + 2, :],
                )

        def prologue_hp(hp):
            """Project + lift K and V for head-pair hp, then transpose V."""
            if hp == 0:
                # critical lead-in: K first, lifted alone on ACT, so the first
                # score MMs only wait for the K chain; V follows.
                project(ksT[:, 0, :], xs, w_sb["wk"], 0, b_sb["bk"][:, 0:1],
                        range(4), copy_on_act=True)
                lift_one_act(ksT, 0)
                project(vT[:, 0, :], xs, w_sb["wv"], 0, b_sb["bv"][:, 0:1],
                        range(4), copy_on_act=True)
                lift_one_act(vT, 0)
                nc.sync.dma_start(v_nat[:, :, 0, :], vT[:, 0, :], transpose=True)
                # prefetch the exp table set while scores are still in flight
                nc.scalar.activation(warm[0:1, 0:16], warm[0:1, 0:16], EXP, scale=0.0)
                return
            project(ksT[:, hp, :], xs, w_sb["wk"], hp, b_sb["bk"][:, hp : hp + 1],
                    range(4))
            project(vT[:, hp, :], xs, w_sb["wv"], hp, b_sb["bv"][:, hp : hp + 1],
                    range(4))
            # 16 time^2 rows (2 proj x 4 chunks x 2 heads) -> one PSUM bank via
            # accumulating mask-variant matmuls (each adds 2 rows + zeros).
            kvnrm = psU.tile([16, 512], F32, tag="u", name="kvnrm")
            nmm = 0
            for pi, src in enumerate((ksT, vT)):
                sq = scr.tile([P, N], BF16, tag="ksq")
                # GpSimd is idle mid-stream; this keeps the DVE free for the
                # Quake lift chains and tail work at head-pair boundaries.
                nc.gpsimd.tensor_tensor(sq[:], src[:, hp, :], src[:, hp, :], MULT)
                for qc in range(4):
                    nc.tensor.matmul(
                        kvnrm[:],
                        lhsT=mask32[:, 4 * pi + qc, 0:16],
                        rhs=sq[:, qc * 512 : (qc + 1) * 512],
                        start=(nmm == 0),
                        stop=(nmm == 7),
                    )
                    nmm += 1
            kvt = scr.tile([16, 512], BF16, tag="kvt")
            lift_times(kvt[:], kvnrm[:], "qk16")
            for pi, dst in enumerate((ksT, vT)):
                for qc in range(4):
                    r = 8 * pi + 2 * qc
                    nc.sync.dma_start(
                        dst[0:65:64, hp, qc * 512 : (qc + 1) * 512],
                        kvt[r : r + 2, :],
                    )
            # V -> natural layout in ONE xbar transpose:
            # transposed row r (= key) lands at v_nat[r%128, r//128, hp, :].
            nc.sync.dma_start(v_nat[:, :, hp, :], vT[:, hp, :], transpose=True)

        def attention_hp(hp):
            pv_tiles[hp] = psPV.tile([P, QB], F32, name=f"pv{hp}", tag=f"pv{hp % 2}")
            for mc in range(16):
                s_ps = psS.tile([P, 1024], F32, tag="s")
                for h in range(2):
                    nc.tensor.matmul(
                        s_ps[:, h * 512 : (h + 1) * 512],
                        lhsT=ksT[h * 64 : (h + 1) * 64, hp, mc * P : (mc + 1) * P],
                        rhs=qsT[h * 64 : (h + 1) * 64, hp, :],
                        start=True,
                        stop=True,
                    )
                p_sb = pP.tile([P, 1024], BF16, tag="p")
                nc.scalar.activation(
                    p_sb[:], s_ps[:], EXP, scale=act_scale, bias=ebias[:]
                )
                for h in range(2):
                    nc.tensor.matmul(
                        pv_tiles[hp][h * 64 : (h + 1) * 64, :],
                        lhsT=v_nat[:, mc, hp, h * 64 : (h + 1) * 64],
                        rhs=p_sb[:, h * 512 : (h + 1) * 512],
                        start=(mc == 0),
                        stop=(mc == 15),
                        skip_group_check=True,
                    )

        o_unT = sb.tile([P, NHP, QB], F32)
        o_nat = sb.tile([P, 4, H, D], F32)  # [q%128, qtile, head, d]
        ph_sq = sb.tile([P, 4, H, D], F32)
        ph_inner = sb.tile([P, 4, H, 1], F32)
        ph_t2 = sb.tile([P, 4, H, 1], F32)

        def tail_hp(hp):
            """Drain PV, transpose to natural layout, DVE part of the per-head
            centroid (no ACT ops -> EXP stream stays uninterrupted)."""
            nc.vector.tensor_copy(out=o_unT[:, hp, :], in_=pv_tiles[hp][:])
            for hh in range(2):
                h = 2 * hp + hh
                for qt in range(4):
                    pt = psU.tile([P, 64], F32, tag="u", name="otp")
                    nc.tensor.transpose(
                        pt[:],
                        o_unT[hh * 64 : (hh + 1) * 64, hp, qt * P : (qt + 1) * P],
                        ident2[hh * 64 : (hh + 1) * 64, :],
                    )
                    nc.vector.tensor_copy(out=o_nat[:, qt, h, :], in_=pt[:])
            hsl = slice(2 * hp, 2 * hp + 2)
            src = o_nat[:, :, hsl, :]
            nc.vector.tensor_tensor(ph_sq[:, :, hsl, :], src, src, MULT)
            nc.vector.tensor_reduce(
                ph_inner[:, :, hsl, 0],
                ph_sq[:, :, hsl, :],
                axis=mybir.AxisListType.X,
                op=ADD,
            )
            nc.vector.tensor_tensor(
                ph_t2[:, :, hsl, :], src[:, :, :, 0:1], src[:, :, :, 0:1], MULT
            )
            nc.vector.tensor_scalar_mul(ph_t2[:, :, hsl, :], ph_t2[:, :, hsl, :], -2.0)
            nc.vector.tensor_tensor(
                ph_inner[:, :, hsl, :], ph_inner[:, :, hsl, :], ph_t2[:, :, hsl, :], ADD
            )

        prologue_hp(0)
        prologue_hp(1)
        attention_hp(0)
        prologue_hp(2)
        attention_hp(1)
        tail_hp(0)
        prologue_hp(3)
        attention_hp(2)
        tail_hp(1)
        attention_hp(3)
        tail_hp(2)
        tail_hp(3)

        # ---- per-head centroid scale, head-sum, final centroid ----
        den = sb.tile([P, 4, H, 1], F32)
        nc.scalar.activation(den[:], ph_inner[:], SQRT, bias=0.0, scale=-1.0)
        rec = sb.tile([P, 4, H, 1], F32)
        nc.vector.reciprocal(rec[:], den[:])
        cent = sb.tile([P, 4, H, D], F32)
        nc.vector.tensor_tensor(
            cent[:], o_nat[:], rec[:].to_broadcast((P, 4, H, D)), MULT
        )
        # head-sum as a contiguous binary tree (strided reduce is slow on DVE)
        c4 = cent[:]  # [P, 4, 8, 64]
        h4 = sb.tile([P, 4, 4, D], F32)
        nc.vector.tensor_tensor(h4[:], c4[:, :, 0:4, :], c4[:, :, 4:8, :], ADD)
        h2 = sb.tile([P, 4, 2, D], F32)
        nc.vector.tensor_tensor(h2[:], h4[:, :, 0:2, :], h4[:, :, 2:4, :], ADD)
        hsum = sb.tile([P, 4, 1, D], F32)
        nc.vector.tensor_tensor(hsum[:], h2[:, :, 0:1, :], h2[:, :, 1:2, :], ADD)
        fsq = sb.tile([P, 4, 1, D], F32)
        nc.vector.tensor_tensor(fsq[:], hsum[:], hsum[:], MULT)
        finner = sb.tile([P, 4, 1, 1], F32)
        nc.vector.tensor_reduce(
            finner[:, :, :, 0], fsq[:], axis=mybir.AxisListType.X, op=ADD
        )
        ft2 = sb.tile([P, 4, 1, 1], F32)
        nc.vector.tensor_tensor(ft2[:], hsum[:, :, :, 0:1], hsum[:, :, :, 0:1], MULT)
        nc.vector.tensor_scalar_mul(ft2[:], ft2[:], -2.0)
        nc.vector.tensor_tensor(finner[:], finner[:], ft2[:], ADD)
        fden = sb.tile([P, 4, 1, 1], F32)
        nc.scalar.activation(fden[:], finner[:], SQRT, bias=0.0, scale=-1.0)
        frec = sb.tile([P, 4, 1, 1], F32)
        nc.vector.reciprocal(frec[:], fden[:])
        out_sb = sb.tile([P, 4, D], F32)
        nc.vector.tensor_tensor(
            out_sb[:],
            hsum[:, :, 0, :],
            frec[:, :, 0, :].to_broadcast((P, 4, D)),
            MULT,
        )
        nc.sync.dma_start(
            io["out"].ap().rearrange("(t p) d -> p t d", p=P), out_sb[:]
        )


def _build(scale_val, bias_val):
    nc = bacc.Bacc(num_devices=N_CORES)
    io = {}
    io["xq_t"] = nc.declare_dram_parameter("xq_t", [P, 4, QB], BF16, isOutput=False)
    for qc in range(4):
        io[f"xs{qc}"] = nc.declare_dram_parameter(
            f"xs{qc}", [P, 4, 512], BF16, isOutput=False
        )
    for nm in ("wq", "wk", "wv"):
        io[nm] = nc.declare_dram_parameter(nm, [P, 4, 4, P], BF16, isOutput=False)
    for nm in ("bq", "bk", "bv"):
        io[nm] = nc.declare_dram_parameter(nm, [P, 4], F32, isOutput=False)
    io["ident2"] = nc.declare_dram_parameter("ident2", [P, 64], F32, isOutput=False)
    io["mask32"] = nc.declare_dram_parameter("mask32", [P, 16, 32], BF16, isOutput=False)
    io["out"] = nc.declare_dram_parameter("out", [QB, D], F32, isOutput=True)

    with tile.TileContext(nc) as tc:
        _emit(tc, nc, io, scale_val, bias_val)
    nc.compile()
    return nc


_BUILD_CACHE = {}


def _get_nc(scale_val, bias_val):
    key = (float(scale_val), float(bias_val))
    if key not in _BUILD_CACHE:
        _BUILD_CACHE[key] = _build(*key)
    return _BUILD_CACHE[key]


def _pad_wT8(w):
    """w: [504, 512] spatial weights for 8 heads -> [512, 512] transposed with
    zero columns at each head's time slot (col h*64)."""
    out = np.zeros((E, 512), dtype=np.float32)
    for h in range(H):
        out[:, h * 64 + 1 : (h + 1) * 64] = w[h * DM1 : (h + 1) * DM1, :].T
    return np.ascontiguousarray(out)


def _pad_b8(b):
    out = np.zeros((512,), dtype=np.float32)
    for h in range(H):
        out[h * 64 + 1 : (h + 1) * 64] = b[h * DM1 : (h + 1) * DM1]
    return out


def _fmt_w(wpad, BF):
    # [E, 512] -> [128 p, 4 ec, 4 pt, 128 m]
    return np.ascontiguousarray(
        wpad.reshape(4, P, 4, P).transpose(1, 0, 2, 3)
    ).astype(BF)


def _fmt_x(x_t, BF):
    # [E, ncols] -> [128 p, 4 ec, ncols]
    return np.ascontiguousarray(
        x_t.reshape(4, P, x_t.shape[1]).transpose(1, 0, 2)
    ).astype(BF)


def make_in_maps(
    query_input, source_input, Wq_w, Wq_b, Wk_w, Wk_b, Wv_w, Wv_b, scale, bias
):
    import ml_dtypes

    BF = ml_dtypes.bfloat16
    ident2 = np.concatenate([np.eye(64), np.eye(64)], axis=0).astype(np.float32)
    mask32 = np.zeros((P, 16, 32), dtype=np.float32)
    for j in range(16):
        mask32[1:64, j, 2 * j] = 1.0
        mask32[65:128, j, 2 * j + 1] = 1.0
    mask32 = mask32.astype(BF)

    wq = _fmt_w(_pad_wT8(Wq_w), BF)
    wk = _fmt_w(_pad_wT8(-Wk_w), BF)  # Lorentz sign folded into K
    wv = _fmt_w(_pad_wT8(Wv_w), BF)
    bq = np.ascontiguousarray(_pad_b8(Wq_b).reshape(4, P).T)
    bk = np.ascontiguousarray(_pad_b8(-Wk_b).reshape(4, P).T)
    bv = np.ascontiguousarray(_pad_b8(Wv_b).reshape(4, P).T)

    xs_chunks = []
    for b in range(B):
        xt = source_input[b].T  # [E, N]
        xs_chunks.append(
            [_fmt_x(xt[:, qc * 512 : (qc + 1) * 512], BF) for qc in range(4)]
        )

    in_maps = []
    for c in range(N_CORES):
        b = c // 4
        g = c % 4
        m = {
            "xq_t": _fmt_x(query_input[b, g * QB : (g + 1) * QB, :].T, BF),
            "wq": wq,
            "wk": wk,
            "wv": wv,
            "bq": bq,
            "bk": bk,
            "bv": bv,
            "ident2": ident2,
            "mask32": mask32,
        }
        for qc in range(4):
            m[f"xs{qc}"] = xs_chunks[b][qc]
        in_maps.append(m)
    return in_maps


def kernel(
    query_input,
    source_input,
    Wq_w,
    Wq_b,
    Wk_w,
    Wk_b,
    Wv_w,
    Wv_b,
    scale,
    bias,
    _trace=False,
):
    scale_val = float(np.asarray(scale).reshape(-1)[0])
    bias_val = float(np.asarray(bias).reshape(-1)[0]) if np.asarray(bias).size else 0.0

    nc = _get_nc(scale_val, bias_val)
    in_maps = make_in_maps(
        query_input, source_input, Wq_w, Wq_b, Wk_w, Wk_b, Wv_w, Wv_b, scale, bias
    )

    from concourse.bass_utils import run_bass_kernel_spmd

    res = run_bass_kernel_spmd(
        nc, in_maps, core_ids=list(range(N_CORES)), trace=_trace
    )

    out = np.zeros((B, N, D), dtype=np.float32)
    for c in range(N_CORES):
        b = c // 4
        g = c % 4
        out[b, g * QB : (g + 1) * QB, :] = res.results[c]["out"]
    if _trace:
        kernel.last_exec_time_ns = res.exec_time_ns
        kernel.last_results = res
    return out


# revision 38
# speedup vs baseline: 1.0862x; 1.0862x over previous
"""Trainium2 Bass kernel for LorentzMultiheadAttention (B=2, N=2048, H=8, D=64, E=512).

Sharding: 8 cores = 2 batches x 4 query-quarters. Core c handles batch b=c//4
and queries [512*(c%4), 512*(c%4+1)) for ALL 8 heads. K/V projections are
recomputed on each core of a batch group (cheaper than an inter-core
ReduceScatter) so the kernel has NO collectives: per-head centroids, the head
mean, and the second centroid are all core-local.

Structure: a 4-stage pipeline over head-pairs. For each head-pair hp:
project K/V -> lift -> one xbar-transpose of V -> 16 attention iterations
(2 score MMs -> EXP[128,1024] -> 2 PV MMs). The next head-pair's projections
are scheduled into the PE slack under the current pair's (ACT-bound) EXP
stream. PSUM: 4 score banks + 2 PV banks (alternating per hp) + 2 utility
banks (projections / lift sums / output transposes) = 8.

ACT table-set discipline: sqrt(x) is computed as exp(0.5*ln(x)) and
1/sqrt(|x|) as exp(-0.5*ln(-x)) so every ACT instruction (lifts, attention
EXPs, centroid scales) draws from the natural_log_exp table set -- no
mid-stream ACT_TABLE_LOADs.

Layout tricks:
- The [128,2048]->[128,16,128] DMA xbar transpose delivers transposed row r
  (= key index) to partition r%128, free tile r//128 -- exactly the
  natural-layout V that PV matmuls need, in ONE instruction per head-pair.
- Lift time rows: accumulating mask-variant matmuls pack all of a head-pair's
  sum(x_spatial^2) rows (K and V, 4 column chunks, even/odd head) into one
  PSUM bank at rows 0..15, so a single 16-lane Ln+Exp computes every
  t = sqrt(1+s); small SBUF->SBUF DMAs scatter the two time rows of each
  target to partitions {0,64}.

Math notes:
- The Lorentz centroid sqrt(C)*x/sqrt(|<x,x>_L|) is scale-invariant, so the
  softmax denominator and the mean-over-heads divide both cancel; PV feeds
  unnormalized sum_m exp(att)*v into the centroid.
- The Lorentz sign is folded by negating K weights on the host:
  scores S' = t_q*t_k - q_s.k_s = -<q,k>_L and softmax weights are
  exp(-(2/s)*S' + (2/s + bias)). No max-subtraction: |att| <= ~3.
"""

import os
import sys

for _p in ("/opt/trn_rl_repo", "/root/.axon_site/_ro/trn_rl_repo"):
    if os.path.isdir(_p) and _p not in sys.path:
        sys.path.insert(0, _p)

import numpy as np

import concourse.bacc as bacc
import concourse.bass as bass
import concourse.mybir as mybir
import concourse.tile as tile

B = 2
N = 2048
H = 8
D = 64
E = 512
DM1 = D - 1  # 63
P = 128
N_CORES = 8
QB = N // 4  # 512 queries per core
NHP = 4  # head-pairs per core

F32 = mybir.dt.float32
BF16 = mybir.dt.bfloat16
I32 = mybir.dt.int32
EXP = mybir.ActivationFunctionType.Exp
SQRT = mybir.ActivationFunctionType.Sqrt
ADD = mybir.AluOpType.add
SUB = mybir.AluOpType.subtract
MULT = mybir.AluOpType.mult
SHR = mybir.AluOpType.logical_shift_right
QMAGIC = 0x5F3759DF


def _emit(tc, nc, io, scale_val, bias_val):
    from contextlib import ExitStack

    ctx = ExitStack()
    with ctx:
        consts = ctx.enter_context(tc.tile_pool(name="consts", bufs=1))
        sb = ctx.enter_context(tc.tile_pool(name="sb", bufs=1))
        scr = ctx.enter_context(tc.tile_pool(name="scr", bufs=2))
        pP = ctx.enter_context(tc.tile_pool(name="pP", bufs=6))
        psU = ctx.enter_context(tc.tile_pool(name="psU", bufs=2, space="PSUM"))
        psPV = ctx.enter_context(tc.tile_pool(name="psPV", bufs=1, space="PSUM"))
        psS = ctx.enter_context(tc.tile_pool(name="psS", bufs=2, space="PSUM"))

        # ---- PE warm-up: the HAM clock-gate starts at 1.2 GHz and needs
        # ~3.4us of sustained matmul activity to reach 2.4 GHz. Input DMAs
        # take ~12us to land, so burn that dead time on scratch matmuls and
        # the real projections start warm.
        warm = sb.tile([P, 512], BF16, name="warm")
        nc.vector.memset(warm[:], 0.5)
        for _ in range(26):
            wps = psU.tile([P, 512], F32, tag="u", name="warmps")
            nc.tensor.matmul(
                wps[:], lhsT=warm[:, 0:P], rhs=warm[:], start=True, stop=True
            )

        # ---- constants / weights (Q-path inputs first so Q proj starts early)
        ident2 = consts.tile([P, 64], F32)
        nc.sync.dma_start(ident2[:], io["ident2"].ap())
        # mask32[:, j, :]: lift-mask variant writing head-sums to rows {2j,2j+1}
        mask32 = consts.tile([P, 16, 32], BF16)
        nc.sync.dma_start(mask32[:], io["mask32"].ap())

        w_sb = {}
        b_sb = {}

        def load_w(nm):
            w = consts.tile([P, 4, 4, P], BF16, name=f"{nm}_sb")
            nc.sync.dma_start(w[:], io[nm].ap())
            w_sb[nm] = w
            bn = "b" + nm[1]
            bt = consts.tile([P, 4], F32, name=f"{bn}_sb")
            nc.sync.dma_start(bt[:], io[bn].ap())
            b_sb[bn] = bt

        load_w("wq")
        xq = sb.tile([P, 4, QB], BF16)
        nc.sync.dma_start(xq[:], io["xq_t"].ap())

        ebias = consts.tile([P, 1], F32)
        nc.vector.memset(ebias[:], 2.0 / scale_val + bias_val)

        qsT = sb.tile([P, NHP, QB], BF16)
        ksT = sb.tile([P, NHP, N], BF16)
        vT = sb.tile([P, NHP, N], BF16)
        v_nat = sb.tile([P, 16, NHP, P], BF16)  # [p, mc, hp, 2h*64]; key=mc*128+p

        IDENT = mybir.ActivationFunctionType.Identity

        def project(dst_sl, x_sl, w, pt, bias, qcs, copy_on_act=False):
            for qc in qcs:
                ps = psU.tile([P, 512], F32, tag="u", name="proj")
                for ec in range(4):
                    nc.tensor.matmul(
                        ps[:],
                        lhsT=w[:, ec, pt, :],
                        rhs=x_sl[:, ec, qc * 512 : (qc + 1) * 512],
                        start=(ec == 0),
                        stop=(ec == 3),
                    )
                dst = dst_sl[:, qc * 512 : (qc + 1) * 512]
                if copy_on_act:
                    # ACT is idle before the EXP stream starts; Identity is in
                    # every table set so this forces no ACT_TABLE_LOAD.
                    nc.scalar.activation(dst, ps[:], IDENT, bias=bias)
                else:
                    nc.vector.tensor_tensor(
                        dst, ps[:], bias.to_broadcast((P, 512)), ADD
                    )

        qmagic = consts.tile([P, 1], I32)
        nc.vector.memset(qmagic[:], QMAGIC)

        def rsqrt_dve(u, tag, iters=1):
            """1/sqrt(u) on the vector engine: Quake seed + Newton steps.
            u: f32 SBUF AP. Returns an f32 tile of the same shape. Keeps the
            ACT engine (and its exp-table) untouched by the lifts."""
            shp = list(u.shape)
            y = scr.tile(shp, F32, tag=f"{tag}y", name="qk_y")
            sh = scr.tile(shp, I32, tag=f"{tag}i", name="qk_i")
            nc.vector.tensor_scalar(sh[:], u.bitcast(I32), 1, None, SHR)
            nc.vector.tensor_tensor(
                y[:].bitcast(I32),
                qmagic[0 : shp[0], :].to_broadcast(tuple(shp)),
                sh[:],
                SUB,
            )
            z = scr.tile(shp, F32, tag=f"{tag}z", name="qk_z")
            for _ in range(iters):
                nc.vector.tensor_tensor(z[:], y[:], y[:], MULT)
                nc.vector.tensor_tensor(z[:], u, z[:], MULT)
                nc.vector.tensor_scalar(z[:], z[:], -0.5, 1.5, MULT, ADD)
                nc.vector.tensor_tensor(y[:], y[:], z[:], MULT)
            return y

        def lift_times(dst, nrm_ps, tag):
            """dst (bf16) = sqrt(1 + nrm_ps) via u*rsqrt(u), DVE-only."""
            shp = list(nrm_ps.shape)
            u = scr.tile(shp, F32, tag=f"{tag}u", name="qk_u")
            nc.vector.tensor_scalar(u[:], nrm_ps, 1.0, None, ADD)
            y = rsqrt_dve(u[:], tag)
            nc.vector.tensor_tensor(dst, u[:], y[:], MULT)

        # ---- Q projection + lift (all 4 head-pairs) ----
        for hp in range(NHP):
            project(
                qsT[:, hp, :], xq, w_sb["wq"], hp, b_sb["bq"][:, hp : hp + 1], [0],
                copy_on_act=True,
            )
        qsq = sb.tile([P, NHP, QB], BF16)
        nc.vector.tensor_tensor(qsq[:], qsT[:], qsT[:], MULT)
        qnrm = psU.tile([8, 512], F32, tag="u", name="qnrm")
        for hp in range(NHP):
            nc.tensor.matmul(
                qnrm[:],
                lhsT=mask32[:, hp, 0:8],
                rhs=qsq[:, hp, :],
                start=(hp == 0),
                stop=(hp == NHP - 1),
            )
        qt_s = scr.tile([8, 512], BF16, tag="qts", bufs=1)
        nc.scalar.activation(qt_s[:], qnrm[:], SQRT, bias=1.0, scale=1.0)
        for hp in range(NHP):
            nc.sync.dma_start(qsT[0:65:64, hp, :], qt_s[2 * hp : 2 * hp + 2, :])

        # source-side inputs (after the Q chain so xq lands first); xs arrives
        # in 4 column chunks so the first K projection starts ASAP.
        load_w("wk")
        load_w("wv")
        xs = sb.tile([P, 4, N], BF16)
        for qc in range(4):
            nc.sync.dma_start(
                xs[:, :, qc * 512 : (qc + 1) * 512], io[f"xs{qc}"].ap()
            )

        act_scale = -2.0 / scale_val
        pv_tiles = {}

        def lift_one_act(srcdst, hp):
            """Pre-stream lift of a single ksT/vT head-pair slice via ACT sqrt
            (ACT is idle before the EXP stream; 0.7us vs ~5.5us DVE Quake)."""
            nrm = psU.tile([8, 512], F32, tag="u", name="nrm")
            sq = scr.tile([P, N], BF16, tag="ksq")
            nc.vector.tensor_tensor(sq[:], srcdst[:, hp, :], srcdst[:, hp, :], MULT)
            for qc in range(4):
                nc.tensor.matmul(
                    nrm[:],
                    lhsT=mask32[:, qc, 0:8],
                    rhs=sq[:, qc * 512 : (qc + 1) * 512],
                    start=(qc == 0),
                    stop=(qc == 3),
                )
            kvt = scr.tile([8, 512], BF16, tag="kvt8")
            nc.scalar.activation(kvt[:], nrm[:], SQRT, bias=1.0, scale=1.0)
            for qc in range(4):
                nc.sync.dma_start(
                    srcdst[0:65:64, hp, qc * 512 : (qc + 1) * 512],
                    kvt[2 * qc : 2 * qc + 2, :],
                )

        def prologue_hp(hp):
            """Project + lift K and V for head-pair hp, then transpose V."""
            if hp == 0:
                # critical lead-in: K first, lifted alone on ACT, so the first
                # score MMs only wait for the K chain; V follows.
                project(ksT[:, 0, :], xs, w_sb["wk"], 0, b_sb["bk"][:, 0:1],
                        range(4), copy_on_act=True)
                lift_one_act(ksT, 0)
                project(vT[:, 0, :], xs, w_sb["wv"], 0, b_sb["bv"][:, 0:1],
                        range(4), copy_on_act=True)
                lift_one_act(vT, 0)
                nc.sync.dma_start(v_nat[:, :, 0, :], vT[:, 0, :], transpose=True)
                # prefetch the exp table set while scores are still in flight
                nc.scalar.activation(warm[0:1, 0:16], warm[0:1, 0:16], EXP, scale=0.0)
                return
            project(ksT[:, hp, :], xs, w_sb["wk"], hp, b_sb["bk"][:, hp : hp + 1],
                    range(4))
            project(vT[:, hp, :], xs, w_sb["wv"], hp, b_sb["bv"][:, hp : hp + 1],
                    range(4))
            # 16 time^2 rows (2 proj x 4 chunks x 2 heads) -> one PSUM bank via
            # accumulating mask-variant matmuls (each adds 2 rows + zeros).
            kvnrm = psU.tile([16, 512], F32, tag="u", name="kvnrm")
            nmm = 0
            for pi, src in enumerate((ksT, vT)):
                sq = scr.tile([P, N], BF16, tag="ksq")
                nc.vector.tensor_tensor(sq[:], src[:, hp, :], src[:, hp, :], MULT)
                for qc in range(4):
                    nc.tensor.matmul(
                        kvnrm[:],
                        lhsT=mask32[:, 4 * pi + qc, 0:16],
                        rhs=sq[:, qc * 512 : (qc + 1) * 512],
                        start=(nmm == 0),
                        stop=(nmm == 7),
                    )
                    nmm += 1
            kvt = scr.tile([16, 512], BF16, tag="kvt")
            lift_times(kvt[:], kvnrm[:], "qk16")
            for pi, dst in enumerate((ksT, vT)):
                for qc in range(4):
                    r = 8 * pi + 2 * qc
                    nc.sync.dma_start(
                        dst[0:65:64, hp, qc * 512 : (qc + 1) * 512],
                        kvt[r : r + 2, :],
                    )
            # V -> natural layout in ONE xbar transpose:
            # transposed row r (= key) lands at v_nat[r%128, r//128, hp, :].
            nc.sync.dma_start(v_nat[:, :, hp, :], vT[:, hp, :], transpose=True)

        def attention_hp(hp):
            pv_tiles[hp] = psPV.tile([P, QB], F32, name=f"pv{hp}", tag=f"pv{hp % 2}")
            for mc in range(16):
                s_ps = psS.tile([P, 1024], F32, tag="s")
                for h in range(2):
                    nc.tensor.matmul(
                        s_ps[:, h * 512 : (h + 1) * 512],
                        lhsT=ksT[h * 64 : (h + 1) * 64, hp, mc * P : (mc + 1) * P],
                        rhs=qsT[h * 64 : (h + 1) * 64, hp, :],
                        start=True,
                        stop=True,
                    )
                p_sb = pP.tile([P, 1024], BF16, tag="p")
                nc.scalar.activation(
                    p_sb[:], s_ps[:], EXP, scale=act_scale, bias=ebias[:]
                )
                for h in range(2):
                    nc.tensor.matmul(
                        pv_tiles[hp][h * 64 : (h + 1) * 64, :],
                        lhsT=v_nat[:, mc, hp, h * 64 : (h + 1) * 64],
                        rhs=p_sb[:, h * 512 : (h + 1) * 512],
                        start=(mc == 0),
                        stop=(mc == 15),
                        skip_group_check=True,
                    )

        o_unT = sb.tile([P, NHP, QB], F32)
        o_nat = sb.tile([P, 4, H, D], F32)  # [q%128, qtile, head, d]
        ph_sq = sb.tile([P, 4, H, D], F32)
        ph_inner = sb.tile([P, 4, H, 1], F32)
        ph_t2 = sb.tile([P, 4, H, 1], F32)

        def tail_hp(hp):
            """Drain PV, transpose to natural layout, DVE part of the per-head
            centroid (no ACT ops -> EXP stream stays uninterrupted)."""
            nc.vector.tensor_copy(out=o_unT[:, hp, :], in_=pv_tiles[hp][:])
            for hh in range(2):
                h = 2 * hp + hh
                for qt in range(4):
                    pt = psU.tile([P, 64], F32, tag="u", name="otp")
                    nc.tensor.transpose(
                        pt[:],
                        o_unT[hh * 64 : (hh + 1) * 64, hp, qt * P : (qt + 1) * P],
                        ident2[hh * 64 : (hh + 1) * 64, :],
                    )
                    nc.vector.tensor_copy(out=o_nat[:, qt, h, :], in_=pt[:])
            hsl = slice(2 * hp, 2 * hp + 2)
            src = o_nat[:, :, hsl, :]
            nc.vector.tensor_tensor(ph_sq[:, :, hsl, :], src, src, MULT)
            nc.vector.tensor_reduce(
                ph_inner[:, :, hsl, 0],
                ph_sq[:, :, hsl, :],
                axis=mybir.AxisListType.X,
                op=ADD,
            )
            nc.vector.tensor_tensor(
                ph_t2[:, :, hsl, :], src[:, :, :, 0:1], src[:, :, :, 0:1], MULT
            )
            nc.vector.tensor_scalar_mul(ph_t2[:, :, hsl, :], ph_t2[:, :, hsl, :], -2.0)
            nc.vector.tensor_tensor(
                ph_inner[:, :, hsl, :], ph_inner[:, :, hsl, :], ph_t2[:, :, hsl, :], ADD
            )

        prologue_hp(0)
        prologue_hp(1)
        attention_hp(0)
        prologue_hp(2)
        attention_hp(1)
        tail_hp(0)
        prologue_hp(3)
        attention_hp(2)
        tail_hp(1)
        attention_hp(3)
        tail_hp(2)
        tail_hp(3)

        # ---- per-head centroid scale, head-sum, final centroid ----
        den = sb.tile([P, 4, H, 1], F32)
        nc.scalar.activation(den[:], ph_inner[:], SQRT, bias=0.0, scale=-1.0)
        rec = sb.tile([P, 4, H, 1], F32)
        nc.vector.reciprocal(rec[:], den[:])
        cent = sb.tile([P, 4, H, D], F32)
        nc.vector.tensor_tensor(
            cent[:], o_nat[:], rec[:].to_broadcast((P, 4, H, D)), MULT
        )
        # head-sum as a contiguous binary tree (strided reduce is slow on DVE)
        c4 = cent[:]  # [P, 4, 8, 64]
        h4 = sb.tile([P, 4, 4, D], F32)
        nc.vector.tensor_tensor(h4[:], c4[:, :, 0:4, :], c4[:, :, 4:8, :], ADD)
        h2 = sb.tile([P, 4, 2, D], F32)
        nc.vector.tensor_tensor(h2[:], h4[:, :, 0:2, :], h4[:, :, 2:4, :], ADD)
        hsum = sb.tile([P, 4, 1, D], F32)
        nc.vector.tensor_tensor(hsum[:], h2[:, :, 0:1, :], h2[:, :, 1:2, :], ADD)
        fsq = sb.tile([P, 4, 1, D], F32)
        nc.vector.tensor_tensor(fsq[:], hsum[:], hsum[:], MULT)
        finner = sb.tile([P, 4, 1, 1], F32)
        nc.vector.tensor_reduce(
            finner[:, :, :, 0], fsq[:], axis=mybir.AxisListType.X, op=ADD
        )
        ft2 = sb.tile([P, 4, 1, 1], F32)
        nc.vector.tensor_tensor(ft2[:], hsum[:, :, :, 0:1], hsum[:, :, :, 0:1], MULT)
        nc.vector.tensor_scalar_mul(ft2[:], ft2[:], -2.0)
        nc.vector.tensor_tensor(finner[:], finner[:], ft2[:], ADD)
        fden = sb.tile([P, 4, 1, 1], F32)
        nc.scalar.activation(fden[:], finner[:], SQRT, bias=0.0, scale=-1.0)
        frec = sb.tile([P, 4, 1, 1], F32)
        nc.vector.reciprocal(frec[:], fden[:])
        out_sb = sb.tile([P, 4, D], F32)
        nc.vector.tensor_tensor(
            out_sb[:],
            hsum[:, :, 0, :],
            frec[:, :, 0, :].to_broadcast((P, 4, D)),
            MULT,
        )
        nc.sync.dma_start(
            io["out"].ap().rearrange("(t p) d -> p t d", p=P), out_sb[:]
        )


def _build(scale_val, bias_val):
    nc = bacc.Bacc(num_devices=N_CORES)
    io = {}
    io["xq_t"] = nc.declare_dram_parameter("xq_t", [P, 4, QB], BF16, isOutput=False)
    for qc in range(4):
        io[f"xs{qc}"] = nc.declare_dram_parameter(
            f"xs{qc}", [P, 4, 512], BF16, isOutput=False
        )
    for nm in ("wq", "wk", "wv"):
        io[nm] = nc.declare_dram_parameter(nm, [P, 4, 4, P], BF16, isOutput=False)
    for nm in ("bq", "bk", "bv"):
        io[nm] = nc.declare_dram_parameter(nm, [P, 4], F32, isOutput=False)
    io["ident2"] = nc.declare_dram_parameter("ident2", [P, 64], F32, isOutput=False)
    io["mask32"] = nc.declare_dram_parameter("mask32", [P, 16, 32], BF16, isOutput=False)
    io["out"] = nc.declare_dram_parameter("out", [QB, D], F32, isOutput=True)

    with tile.TileContext(nc) as tc:
        _emit(tc, nc, io, scale_val, bias_val)
    nc.compile()
    return nc


_BUILD_CACHE = {}


def _get_nc(scale_val, bias_val):
    key = (float(scale_val), float(bias_val))
    if key not in _BUILD_CACHE:
        _BUILD_CACHE[key] = _build(*key)
    return _BUILD_CACHE[key]


def _pad_wT8(w):
    """w: [504, 512] spatial weights for 8 heads -> [512, 512] transposed with
    zero columns at each head's time slot (col h*64)."""
    out = np.zeros((E, 512), dtype=np.float32)
    for h in range(H):
        out[:, h * 64 + 1 : (h + 1) * 64] = w[h * DM1 : (h + 1) * DM1, :].T
    return np.ascontiguousarray(out)


def _pad_b8(b):
    out = np.zeros((512,), dtype=np.float32)
    for h in range(H):
        out[h * 64 + 1 : (h + 1) * 64] = b[h * DM1 : (h + 1) * DM1]
    return out


def _fmt_w(wpad, BF):
    # [E, 512] -> [128 p, 4 ec, 4 pt, 128 m]
    return np.ascontiguousarray(
        wpad.reshape(4, P, 4, P).transpose(1, 0, 2, 3)
    ).astype(BF)


def _fmt_x(x_t, BF):
    # [E, ncols] -> [128 p, 4 ec, ncols]
    return np.ascontiguousarray(
        x_t.reshape(4, P, x_t.shape[1]).transpose(1, 0, 2)
    ).astype(BF)


def make_in_maps(
    query_input, source_input, Wq_w, Wq_b, Wk_w, Wk_b, Wv_w, Wv_b, scale, bias
):
    import ml_dtypes

    BF = ml_dtypes.bfloat16
    ident2 = np.concatenate([np.eye(64), np.eye(64)], axis=0).astype(np.float32)
    mask32 = np.zeros((P, 16, 32), dtype=np.float32)
    for j in range(16):
        mask32[1:64, j, 2 * j] = 1.0
        mask32[65:128, j, 2 * j + 1] = 1.0
    mask32 = mask32.astype(BF)

    wq = _fmt_w(_pad_wT8(Wq_w), BF)
    wk = _fmt_w(_pad_wT8(-Wk_w), BF)  # Lorentz sign folded into K
    wv = _fmt_w(_pad_wT8(Wv_w), BF)
    bq = np.ascontiguousarray(_pad_b8(Wq_b).reshape(4, P).T)
    bk = np.ascontiguousarray(_pad_b8(-Wk_b).reshape(4, P).T)
    bv = np.ascontiguousarray(_pad_b8(Wv_b).reshape(4, P).T)

    xs_chunks = []
    for b in range(B):
        xt = source_input[b].T  # [E, N]
        xs_chunks.append(
            [_fmt_x(xt[:, qc * 512 : (qc + 1) * 512], BF) for qc in range(4)]
        )

    in_maps = []
    for c in range(N_CORES):
        b = c // 4
        g = c % 4
        m = {
            "xq_t": _fmt_x(query_input[b, g * QB : (g + 1) * QB, :].T, BF),
            "wq": wq,
            "wk": wk,
            "wv": wv,
            "bq": bq,
            "bk": bk,
            "bv": bv,
            "ident2": ident2,
            "mask32": mask32,
        }
        for qc in range(4):
            m[f"xs{qc}"] = xs_chunks[b][qc]
        in_maps.append(m)
    return in_maps


def kernel(
    query_input,
    source_input,
    Wq_w,
    Wq_b,
    Wk_w,
    Wk_b,
    Wv_w,
    Wv_b,
    scale,
    bias,
    _trace=False,
):
    scale_val = float(np.asarray(scale).reshape(-1)[0])
    bias_val = float(np.asarray(bias).reshape(-1)[0]) if np.asarray(bias).size else 0.0

    nc = _get_nc(scale_val, bias_val)
    in_maps = make_in_maps(
        query_input, source_input, Wq_w, Wq_b, Wk_w, Wk_b, Wv_w, Wv_b, scale, bias
    )

    from concourse.bass_utils import run_bass_kernel_spmd

    res = run_bass_kernel_spmd(
        nc, in_maps, core_ids=list(range(N_CORES)), trace=_trace
    )

    out = np.zeros((B, N, D), dtype=np.float32)
    for c in range(N_CORES):
        b = c // 4
        g = c % 4
        out[b, g * QB : (g + 1) * QB, :] = res.results[c]["out"]
    if _trace:
        kernel.last_exec_time_ns = res.exec_time_ns
        kernel.last_results = res
    return out
